# revision 20
# baseline (speedup 1.0000x reference)
"""Trainium2 Bass kernel for a 3D attention block.

Reference computation (per batch b):
    xf = x[b].reshape(C, N)                       # C=256, N=4096
    q  = Wq @ xf + bq                             # [32, N]
    k  = Wk @ xf + bk                             # [32, N]
    v  = Wv @ xf + bv                             # [256, N]
    P  = softmax(q.T @ k, axis=-1)                # [N(m), N(n)]
    out[c, m] = sum_n v[c, n] * P[m, n]
    result = gamma * out + x[b]

Sharding: 8 cores = 2 batches x 4 chunks of 1024 query rows (m).
SPMD trick: every core receives x pre-rolled along n by -1024*j so its
query chunk sits at columns 0:1024.  Softmax rowsum and PV are
permutation-invariant in n, so k/v simply use the rolled order and no
per-core program differences are needed.

On-device layout (per core) is transpose-free:
    S^T[n, m] = k^T q   (n on partitions)  -> exp on ACT -> P^T in SBUF
    out[c, m] = sum over n-tiles of vT[n-tile, c].T @ P^T[n-tile, m]
Softmax max-subtraction is skipped (|S| <= ~25, exp stays in fp32/bf16
range).

Perf notes (hard-won on this device):
  - The PE p-state collapses to ~1.2GHz whenever its instruction stream
    has per-tile gaps; a proj-first structure or a <=4-matmul/tile loop
    both trigger it (~600ns per 512-col matmul instead of ~380ns).  The
    projections are therefore interleaved with the attention loop and
    each tile issues 6 matmuls (S^T x2 + PV x4) to keep the PE
    backlogged.
  - Rowsum runs off-PE: P^T tiles are chain-accumulated on DVE in bf16
    (4 chains + tree), one ones^T matmul pair finishes it.  Saves 64
    matmuls / 32k PE cycles vs rowsum-by-matmul.
  - v projection is one fp8e4 DoubleRow matmul per n-tile (contracts
    both 128-halves of C at once).  Host-validated absmax ~6e-2 on an
    output scale of 5.3 (rel ~1.2e-2 < 2e-2 gate); q/k must stay fp16
    (fp8 there gives absmax 0.35 through the softmax).
  - Residual uses the fp16 x directly (no separate fp32 residual DMA).

ATTN_KERNEL_REPEATS=<R> emits the body R times in one NEFF (timing via
slope; outputs are idempotent). ATTN_KERNEL_TRACE=1 captures an NTFF
profile via run_bass_kernel_spmd(trace=True).
ATTN_V_FP8=0 falls back to an fp16 v projection (2 matmuls/tile).
"""

import os

import numpy as np

import concourse.bass as bass
import concourse.mybir as mybir
import concourse.tile as tile
from concourse import bacc
from concourse.bass_utils import run_bass_kernel_spmd

F32 = mybir.dt.float32
F16 = mybir.dt.float16
BF16 = mybir.dt.bfloat16
F8 = mybir.dt.float8e4

C = 256
C8 = 32
N = 4096  # 16*16*16 voxels
MCHUNK = 1024  # query rows per core
NT = N // 128  # 32 key tiles
NCORES = 8
V_FP8 = int(os.environ.get("ATTN_V_FP8", "1"))

# info stashed by the last kernel() call (for test harnesses)
LAST_RESULTS = None


def _emit_body(nc, tc, io, rep):
    xf16, x8, wqk, wv8, bqk, bv, gamma, out = io
    r = f"_{rep}"
    with (
        tc.tile_pool(name="big" + r, bufs=1) as big,
        tc.tile_pool(name="ptp" + r, bufs=4) as ptp,
        tc.tile_pool(name="epi" + r, bufs=2) as epi,
        tc.tile_pool(name="pacc" + r, bufs=1, space="PSUM") as pacc,
        tc.tile_pool(name="pst" + r, bufs=2, space="PSUM") as pst,
    ):
        def chunk_sl(ch):
            return slice(ch * 512, (ch + 1) * 512)

        # ---- input DMAs.  Weights first, then x chunks in consumption
        # order; small constants go on the gpsimd queue so they don't
        # delay the x stream on the sync queue.
        wqk_t = big.tile([128, 2, 2 * C8], F16, name="wqk_t" + r)
        nc.sync.dma_start(wqk_t[:], wqk[:])
        xf_t = big.tile([128, 2, N], F16, name="xf_t" + r)
        x8_t = None
        if V_FP8:
            x8_t = big.tile([128, 2, N], F8, name="x8_t" + r)
            wv8_t = big.tile([128, 2, C], F8, name="wv8_t" + r)
            nc.sync.dma_start(wv8_t[:], wv8[:])
        else:
            wv_t = big.tile([128, 2, C], F16, name="wv_t" + r)
            nc.sync.dma_start(wv_t[:], wv8[:])

        # first two chunks feed q; ship them before everything else
        for ch in range(2):
            nc.sync.dma_start(xf_t[:, :, chunk_sl(ch)], xf16[:, :, chunk_sl(ch)])

        bqk_t = big.tile([C8, 2], F32, name="bqk_t" + r)
        nc.gpsimd.dma_start(bqk_t[:], bqk[:])
        bv_b = big.tile([128, C], F32, name="bv_b" + r)
        nc.gpsimd.dma_start(
            bv_b[:], bass.AP(tensor=bv, offset=0, ap=[[0, 128], [1, C]])
        )
        gamma_b = big.tile([128, 1], F32, name="gamma_b" + r)
        nc.gpsimd.dma_start(
            gamma_b[:], bass.AP(tensor=gamma, offset=0, ap=[[0, 128], [1, 1]])
        )

        if V_FP8:
            nc.sync.dma_start(x8_t[:, :, 0:1024], x8[:, :, 0:1024])
        for ch in range(2, 8):
            nc.sync.dma_start(xf_t[:, :, chunk_sl(ch)], xf16[:, :, chunk_sl(ch)])
            if V_FP8 and ch % 2 == 1:
                sl2 = slice((ch - 1) * 512, (ch + 1) * 512)
                nc.sync.dma_start(x8_t[:, :, sl2], x8[:, :, sl2])

        bq_t = bqk_t[:, 0:1]
        bk_t = bqk_t[:, 1:2]
        wu = big.tile([128, 512], BF16, name="wu" + r)
        nc.vector.memset(wu[:], 0.0)
        ones_t = big.tile([128, 1], BF16, name="ones_t" + r)
        nc.vector.memset(ones_t[:], 1.0)
        ones_row = big.tile([1, 128], F32, name="ones_row" + r)
        nc.vector.memset(ones_row[:], 1.0)

        q_sb = big.tile([C8, MCHUNK], F16, name="q_sb" + r)
        k_sb = big.tile([C8, N], F16, name="k_sb" + r)
        vt_sb = big.tile([128, NT, C], BF16, name="vt_sb" + r)

        # acc[h] accumulates out[c-half, m] across the whole loop
        acc = [pacc.tile([128, MCHUNK], F32, name=f"acc{h}" + r) for h in range(2)]

        # warm-up matmuls on zeros while the x stream is still in
        # flight: keeps the PE executing through the DMA lead so its
        # p-state is ramped when the real projections start (acc is
        # reset by PV's start=True later)
        for w in range(8):
            nc.tensor.matmul(
                acc[w % 2][:, 0:512], wu[:, 0:128], wu[:], start=True, stop=True
            )

        # ---- q projection (needs chunks 0,1 of xf) ----
        for mh in range(2):
            sl = chunk_sl(mh)
            q_ps = pst.tile([128, MCHUNK], F32, tag="st", name=f"q_ps{mh}" + r)
            nc.tensor.matmul(
                q_ps[:C8, 0:512], wqk_t[:, 0, 0:C8], xf_t[:, 0, sl],
                start=True, stop=False,
            )
            nc.tensor.matmul(
                q_ps[:C8, 0:512], wqk_t[:, 1, 0:C8], xf_t[:, 1, sl],
                start=False, stop=True,
            )
            nc.vector.tensor_scalar_add(q_sb[:, sl], q_ps[:C8, 0:512], bq_t)

        def emit_proj_chunk(ch):
            # one pst tile for k, one for all four v quarters: 2 pool
            # allocations per chunk instead of 5, so the S^T tiles keep
            # nearly the full 2-slot double-buffering depth
            sl = chunk_sl(ch)
            k_ps = pst.tile([128, MCHUNK], F32, tag="st", name=f"k_ps{ch}" + r)
            nc.tensor.matmul(
                k_ps[:C8, 0:512], wqk_t[:, 0, C8 : 2 * C8], xf_t[:, 0, sl],
                start=True, stop=False,
            )
            nc.tensor.matmul(
                k_ps[:C8, 0:512], wqk_t[:, 1, C8 : 2 * C8], xf_t[:, 1, sl],
                start=False, stop=True,
            )
            nc.vector.tensor_scalar_add(k_sb[:, sl], k_ps[:C8, 0:512], bk_t)
            v_ps = pst.tile([128, MCHUNK], F32, tag="st", name=f"v_ps{ch}" + r)
            for q4 in range(4):
                nt = 4 * ch + q4
                ntsl = slice(nt * 128, (nt + 1) * 128)
                csl = slice(q4 * C, (q4 + 1) * C)
                if V_FP8:
                    nc.tensor.matmul(
                        v_ps[:, csl], x8_t[:, :, ntsl], wv8_t[:],
                        start=True, stop=True,
                        perf_mode=mybir.MatmulPerfMode.DoubleRow,
                    )
                else:
                    nc.tensor.matmul(
                        v_ps[:, csl], xf_t[:, 0, ntsl], wv_t[:, 0, :],
                        start=True, stop=False,
                    )
                    nc.tensor.matmul(
                        v_ps[:, csl], xf_t[:, 1, ntsl], wv_t[:, 1, :],
                        start=False, stop=True,
                    )
                nc.vector.tensor_add(vt_sb[:, nt, :], v_ps[:, csl], bv_b[:])

        # ---- main attention loop ----
        # pac[j] accumulate P^T tiles off-PE (for the softmax rowsum)
        pac = [big.tile([128, MCHUNK], BF16, name=f"pac{j}" + r) for j in range(4)]

        pts = [None] * NT

        def emit_st_exp(nt):
            ksl = k_sb[:, nt * 128 : (nt + 1) * 128]
            st = pst.tile([128, MCHUNK], F32, tag="st", name=f"st{nt}" + r)
            pt = ptp.tile([128, MCHUNK], BF16, tag="pt", name=f"pt{nt}" + r)
            for mh in range(2):
                msl = slice(mh * 512, (mh + 1) * 512)
                nc.tensor.matmul(
                    st[:, msl], ksl, q_sb[:, msl], start=True, stop=True
                )
            nc.scalar.activation(pt[:], st[:], mybir.ActivationFunctionType.Exp)
            pts[nt] = pt
            # P-accumulation for the rowsum: chains 0,1 on DVE and 2,3 on
            # GPSIMD, so the DVE queue stays short (its latency feeds the
            # S^T psum-slot recycling chain).  The final two tiles go to
            # DVE regardless: a slow GPSIMD add there would gate the
            # rowsum tree at the tail.
            eng = nc.vector if (nt % 4 < 2 or nt >= NT - 2) else nc.gpsimd
            if nt < 4:
                eng.tensor_copy(pac[nt], pt[:])
            else:
                eng.tensor_add(pac[nt % 4], pac[nt % 4], pt[:])
            if nt == NT - 3:
                # chains 0,1 complete at tiles 28,29; fold them early
                nc.vector.tensor_add(pac[0], pac[0], pac[1])

        def emit_pv(nt):
            first, last = nt == 0, nt == NT - 1
            pt = pts[nt]
            for h in range(2):
                vsl = vt_sb[:, nt, h * 128 : (h + 1) * 128]
                for mh in range(2):
                    msl = slice(mh * 512, (mh + 1) * 512)
                    nc.tensor.matmul(
                        acc[h][:, msl], vsl, pt[:, msl], start=first, stop=last
                    )

        # chunk-phased emission: project chunk i while attending chunk i-1
        LAG = 2
        for i in range(9):
            if i < 8:
                emit_proj_chunk(i)
            if i >= 1:
                for nt in range(4 * (i - 1), 4 * i):
                    emit_st_exp(nt)
                    if nt >= LAG:
                        emit_pv(nt - LAG)
        for nt in range(NT - LAG, NT):
            emit_pv(nt)

        # finish the rowsum tree on DVE (pac0+pac1 was folded in-loop)
        nc.vector.tensor_add(pac[2], pac[2], pac[3])
        nc.vector.tensor_add(pac[0], pac[0], pac[2])
        rs_tile = pst.tile([128, MCHUNK], F32, tag="st", name="rs_t" + r)
        rs_ps = rs_tile[:1, :]
        for mh in range(2):
            msl = slice(mh * 512, (mh + 1) * 512)
            nc.tensor.matmul(
                rs_ps[:, msl], ones_t[:], pac[0][:, msl], start=True, stop=True
            )

        # ---- epilogue: out = acc * (gamma / rowsum) + x ----
        # recip on DVE, fp32 broadcast matmul on (idle) PE, gamma folded
        # into the ACT copy off PSUM, final mul/add split DVE / GPSIMD.
        rinv = epi.tile([1, MCHUNK], F32, name="rinv" + r)
        grecip_b = big.tile([128, MCHUNK], F32, name="gr_b" + r)
        res = [
            epi.tile([128, MCHUNK], F32, tag=f"res{h}", name=f"res{h}" + r)
            for h in range(2)
        ]
        gr_tile = pst.tile([128, MCHUNK], F32, tag="st", name="gr_ps" + r)
        for mh in range(2):
            msl = slice(mh * 512, (mh + 1) * 512)
            nc.vector.reciprocal_approx_fast(rinv[:, msl], rs_ps[:, msl])
            nc.tensor.matmul(
                gr_tile[:, msl], ones_row[:], rinv[:, msl], start=True, stop=True
            )
            nc.scalar.activation(
                grecip_b[:, msl], gr_tile[:, msl],
                mybir.ActivationFunctionType.Copy, scale=gamma_b[:],
            )
            # GPSIMD cannot read PSUM: both muls (PSUM src) stay on DVE,
            # then the all-SBUF residual adds run DVE/GPSIMD in parallel.
            for h in range(2):
                nc.vector.tensor_mul(res[h][:, msl], acc[h][:, msl], grecip_b[:, msl])
            for h in range(2):
                eng = nc.vector if h == 0 else nc.gpsimd
                eng.tensor_add(res[h][:, msl], res[h][:, msl], xf_t[:, h, msl])
                nc.sync.dma_start(out[:, h, msl], res[h][:, msl])


def _build(repeats=1):
    nc = bacc.Bacc("TRN2", target_bir_lowering=False, debug=False, num_devices=NCORES)

    xf16 = nc.dram_tensor("xf16", [128, 2, N], F16, kind="ExternalInput")
    x8 = nc.dram_tensor("x8", [128, 2, N], F8 if V_FP8 else F16, kind="ExternalInput")
    wqk = nc.dram_tensor("wqk", [128, 2, 2 * C8], F16, kind="ExternalInput")
    wv8 = nc.dram_tensor(
        "wv8", [128, 2, C], F8 if V_FP8 else F16, kind="ExternalInput"
    )
    bqk = nc.dram_tensor("bqk", [C8, 2], F32, kind="ExternalInput")
    bv = nc.dram_tensor("bv", [1, C], F32, kind="ExternalInput")
    gamma = nc.dram_tensor("gamma", [1, 1], F32, kind="ExternalInput")
    out = nc.dram_tensor("out", [128, 2, MCHUNK], F32, kind="ExternalOutput")
    io = (xf16, x8, wqk, wv8, bqk, bv, gamma, out)

    with tile.TileContext(nc) as tc:
        for rep in range(repeats):
            _emit_body(nc, tc, io, rep)

    nc.compile()
    return nc


_NC_CACHE = {}


def _get_nc(repeats=1):
    if repeats not in _NC_CACHE:
        _NC_CACHE[repeats] = _build(repeats)
    return _NC_CACHE[repeats]


def _in_maps(x, Wq, bq, Wk, bk, Wv, bv, gamma):
    import ml_dtypes

    f8dt = ml_dtypes.float8_e4m3 if V_FP8 else np.float16
    xflat = x.reshape(2, C, N)
    # [C, N] -> [128, 2, N] with c = h*128 + p
    xh16 = xflat.astype(np.float16).reshape(2, 2, 128, N).transpose(0, 2, 1, 3)
    xh8 = xflat.astype(f8dt).reshape(2, 2, 128, N).transpose(0, 2, 1, 3)
    # [128, 2, 64]: wqk[p, h, o] = [Wq.T | Wk.T][h*128+p, o]
    wqk_full = np.concatenate([Wq.T, Wk.T], axis=1).astype(np.float16)  # [C, 64]
    wqk2 = np.ascontiguousarray(wqk_full.reshape(2, 128, 2 * C8).transpose(1, 0, 2))
    # [128, 2, 256]: wv8[p, h, co] = Wv[co, h*128+p]
    wv82 = np.ascontiguousarray(
        Wv.T.astype(f8dt).reshape(2, 128, C).transpose(1, 0, 2)
    )
    bqk2 = np.ascontiguousarray(
        np.stack([bq.reshape(C8), bk.reshape(C8)], axis=1).astype(np.float32)
    )
    bv2 = np.ascontiguousarray(bv.reshape(1, C))
    g2 = np.ascontiguousarray(gamma.reshape(1, 1))

    maps = []
    for core in range(NCORES):
        b, j = core // 4, core % 4
        roll = -j * MCHUNK
        maps.append(
            {
                "xf16": np.ascontiguousarray(np.roll(xh16[b], roll, axis=2)),
                "x8": np.ascontiguousarray(np.roll(xh8[b], roll, axis=2)),
                "wqk": wqk2,
                "wv8": wv82,
                "bqk": bqk2,
                "bv": bv2,
                "gamma": g2,
            }
        )
    return maps


def kernel(x, Wq, bq, Wk, bk, Wv, bv, gamma):
    global LAST_RESULTS
    x = np.ascontiguousarray(np.asarray(x, dtype=np.float32))
    args = [np.asarray(a, dtype=np.float32) for a in (Wq, bq, Wk, bk, Wv, bv, gamma)]

    B, Cc, D, H, W = x.shape
    assert (B, Cc, D * H * W) == (2, C, N), x.shape

    repeats = int(os.environ.get("ATTN_KERNEL_REPEATS", "1"))
    nc = _get_nc(repeats)
    maps = _in_maps(x, *args)
    kwargs = {}
    if int(os.environ.get("ATTN_KERNEL_TRACE", "0")):
        kwargs = dict(
            trace=True,
            trace_cores=[0],
            tmpdir=os.environ.get("ATTN_KERNEL_TRACE_DIR"),
        )
    res = run_bass_kernel_spmd(nc, maps, core_ids=list(range(NCORES)), **kwargs)
    LAST_RESULTS = res

    outf = np.empty((B, C, N), dtype=np.float32)
    for core in range(NCORES):
        b, j = core // 4, core % 4
        o = res.results[core]["out"]  # [128, 2, 1024]
        outf[b][:, j * MCHUNK : (j + 1) * MCHUNK] = o.transpose(1, 0, 2).reshape(
            C, MCHUNK
        )
    return outf.reshape(B, Cc, D, H, W)


# revision 38
# speedup vs baseline: 1.2170x; 1.2170x over previous
"""Trainium2 Bass kernel for a 3D attention block.

Reference computation (per batch b):
    xf = x[b].reshape(C, N)                       # C=256, N=4096
    q  = Wq @ xf + bq                             # [32, N]
    k  = Wk @ xf + bk                             # [32, N]
    v  = Wv @ xf + bv                             # [256, N]
    P  = softmax(q.T @ k, axis=-1)                # [N(m), N(n)]
    out[c, m] = sum_n v[c, n] * P[m, n]
    result = gamma * out + x[b]

Sharding: 8 cores = 2 batches x 4 chunks of 1024 query rows (m).
SPMD trick: every core receives x pre-rolled along n by -1024*j so its
query chunk sits at columns 0:1024.  Softmax rowsum and PV are
permutation-invariant in n, so k/v simply use the rolled order and no
per-core program differences are needed.

On-device layout (per core) is transpose-free:
    S^T[n, m] = k^T q   (n on partitions)  -> exp on ACT -> P^T in SBUF
    out[c, m] = sum over n-tiles of vT[n-tile, c].T @ P^T[n-tile, m]
Softmax max-subtraction is skipped (|S| <= ~25, exp stays in fp32/bf16
range).

Perf notes (hard-won on this device):
  - The PE p-state collapses to ~1.2GHz whenever its instruction stream
    has per-tile gaps; a proj-first structure or a <=4-matmul/tile loop
    both trigger it (~600ns per 512-col matmul instead of ~380ns).  The
    projections are therefore interleaved with the attention loop and
    each tile issues 6 matmuls (S^T x2 + PV x4) to keep the PE
    backlogged.
  - Rowsum runs off-PE: P^T tiles are chain-accumulated on DVE in bf16
    (4 chains + tree), one ones^T matmul pair finishes it.  Saves 64
    matmuls / 32k PE cycles vs rowsum-by-matmul.
  - v projection is one fp8e4 DoubleRow matmul per n-tile (contracts
    both 128-halves of C at once).  Host-validated absmax ~6e-2 on an
    output scale of 5.3 (rel ~1.2e-2 < 2e-2 gate); q/k must stay fp16
    (fp8 there gives absmax 0.35 through the softmax).
  - Residual uses the fp16 x directly (no separate fp32 residual DMA).

ATTN_KERNEL_REPEATS=<R> emits the body R times in one NEFF (timing via
slope; outputs are idempotent). ATTN_KERNEL_TRACE=1 captures an NTFF
profile via run_bass_kernel_spmd(trace=True).
ATTN_V_FP8=0 falls back to an fp16 v projection (2 matmuls/tile).
"""

import os

import numpy as np

import concourse.bass as bass
import concourse.mybir as mybir
import concourse.tile as tile
from concourse import bacc
from concourse.bass_utils import run_bass_kernel_spmd

F32 = mybir.dt.float32
F16 = mybir.dt.float16
BF16 = mybir.dt.bfloat16
F8 = mybir.dt.float8e4

C = 256
C8 = 32
N = 4096  # 16*16*16 voxels
MCHUNK = 1024  # query rows per core
NT = N // 128  # 32 key tiles
NCORES = 8
V_FP8 = int(os.environ.get("ATTN_V_FP8", "1"))

# info stashed by the last kernel() call (for test harnesses)
LAST_RESULTS = None


def _emit_body(nc, tc, io, rep):
    xf16, x8, wqk, wv8, bqk, bv, gamma, out = io
    r = f"_{rep}"
    with (
        tc.tile_pool(name="big" + r, bufs=1) as big,
        tc.tile_pool(name="ptp" + r, bufs=4) as ptp,
        tc.tile_pool(name="epi" + r, bufs=2) as epi,
        tc.tile_pool(name="pacc" + r, bufs=1, space="PSUM") as pacc,
        tc.tile_pool(name="pst" + r, bufs=2, space="PSUM") as pst,
    ):
        def chunk_sl(ch):
            return slice(ch * 512, (ch + 1) * 512)

        # ---- input DMAs.  Weights first, then x chunks in consumption
        # order; small constants go on the gpsimd queue so they don't
        # delay the x stream on the sync queue.
        wqk_t = big.tile([128, 2, 2 * C8], F16, name="wqk_t" + r)
        nc.sync.dma_start(wqk_t[:], wqk[:])
        xf_t = big.tile([128, 2, N], F16, name="xf_t" + r)
        x8_t = None
        # first two chunks feed q; ship them before everything else
        for ch in range(2):
            nc.sync.dma_start(xf_t[:, :, chunk_sl(ch)], xf16[:, :, chunk_sl(ch)])
        if V_FP8:
            x8_t = big.tile([128, 2, N], F8, name="x8_t" + r)
            wv8_t = big.tile([128, 2, C], F8, name="wv8_t" + r)
            nc.sync.dma_start(wv8_t[:], wv8[:])
        else:
            wv_t = big.tile([128, 2, C], F16, name="wv_t" + r)
            nc.sync.dma_start(wv_t[:], wv8[:])

        bqk_t = big.tile([C8, 2], F32, name="bqk_t" + r)
        nc.gpsimd.dma_start(bqk_t[:], bqk[:])
        bv_b = big.tile([128, C], F32, name="bv_b" + r)
        nc.gpsimd.dma_start(
            bv_b[:], bass.AP(tensor=bv, offset=0, ap=[[0, 128], [1, C]])
        )
        gamma_b = big.tile([128, 1], F32, name="gamma_b" + r)
        nc.gpsimd.dma_start(
            gamma_b[:], bass.AP(tensor=gamma, offset=0, ap=[[0, 128], [1, 1]])
        )
        # 1/gamma as the rowsum matmul's stationary vector: the rowsum
        # comes out pre-divided by gamma, so recip directly yields
        # gamma/rowsum and no separate gamma scaling stage is needed
        ginv_f = big.tile([128, 1], F32, name="ginv_f" + r)
        nc.vector.reciprocal(ginv_f[:], gamma_b[:])
        ginv_b = big.tile([128, 1], BF16, name="ginv_b" + r)
        nc.vector.tensor_copy(ginv_b[:], ginv_f[:])

        if V_FP8:
            nc.sync.dma_start(x8_t[:, :, 0:1024], x8[:, :, 0:1024])
        for ch in range(2, 8):
            nc.sync.dma_start(xf_t[:, :, chunk_sl(ch)], xf16[:, :, chunk_sl(ch)])
            if V_FP8 and ch % 2 == 1:
                sl2 = slice((ch - 1) * 512, (ch + 1) * 512)
                nc.sync.dma_start(x8_t[:, :, sl2], x8[:, :, sl2])

        bq_t = bqk_t[:, 0:1]
        bk_t = bqk_t[:, 1:2]
        wu = big.tile([128, 512], BF16, name="wu" + r)
        nc.vector.memset(wu[:], 0.0)
        ones_row = big.tile([1, 128], F32, name="ones_row" + r)
        nc.vector.memset(ones_row[:], 1.0)

        q_sb = big.tile([C8, MCHUNK], F16, name="q_sb" + r)
        k_sb = big.tile([C8, N], F16, name="k_sb" + r)
        vt_sb = big.tile([128, NT, C], BF16, name="vt_sb" + r)

        # acc[h] accumulates out[c-half, m] across the whole loop
        acc = [pacc.tile([128, MCHUNK], F32, name=f"acc{h}" + r) for h in range(2)]

        # warm-up matmuls on zeros while the x stream is still in
        # flight: keeps the PE executing through the DMA lead so its
        # p-state is ramped when the real projections start (acc is
        # reset by PV's start=True later)
        for w in range(8):
            nc.tensor.matmul(
                acc[w % 2][:, 0:512], wu[:, 0:128], wu[:], start=True, stop=True
            )

        # ---- q projection (needs chunks 0,1 of xf) ----
        for mh in range(2):
            sl = chunk_sl(mh)
            q_ps = pst.tile([128, MCHUNK], F32, tag="st", name=f"q_ps{mh}" + r)
            nc.tensor.matmul(
                q_ps[:C8, 0:512], wqk_t[:, 0, 0:C8], xf_t[:, 0, sl],
                start=True, stop=False,
            )
            nc.tensor.matmul(
                q_ps[:C8, 0:512], wqk_t[:, 1, 0:C8], xf_t[:, 1, sl],
                start=False, stop=True,
            )
            nc.vector.tensor_scalar_add(q_sb[:, sl], q_ps[:C8, 0:512], bq_t)

        def emit_proj_chunk(ch):
            # one pst tile for k, one for all four v quarters: 2 pool
            # allocations per chunk instead of 5, so the S^T tiles keep
            # nearly the full 2-slot double-buffering depth
            sl = chunk_sl(ch)
            k_ps = pst.tile([128, MCHUNK], F32, tag="st", name=f"k_ps{ch}" + r)
            nc.tensor.matmul(
                k_ps[:C8, 0:512], wqk_t[:, 0, C8 : 2 * C8], xf_t[:, 0, sl],
                start=True, stop=False,
            )
            nc.tensor.matmul(
                k_ps[:C8, 0:512], wqk_t[:, 1, C8 : 2 * C8], xf_t[:, 1, sl],
                start=False, stop=True,
            )
            nc.vector.tensor_scalar_add(k_sb[:, sl], k_ps[:C8, 0:512], bk_t)
            v_ps = pst.tile([128, MCHUNK], F32, tag="st", name=f"v_ps{ch}" + r)
            for q4 in range(4):
                nt = 4 * ch + q4
                ntsl = slice(nt * 128, (nt + 1) * 128)
                csl = slice(q4 * C, (q4 + 1) * C)
                if V_FP8:
                    nc.tensor.matmul(
                        v_ps[:, csl], x8_t[:, :, ntsl], wv8_t[:],
                        start=True, stop=True,
                        perf_mode=mybir.MatmulPerfMode.DoubleRow,
                    )
                else:
                    nc.tensor.matmul(
                        v_ps[:, csl], xf_t[:, 0, ntsl], wv_t[:, 0, :],
                        start=True, stop=False,
                    )
                    nc.tensor.matmul(
                        v_ps[:, csl], xf_t[:, 1, ntsl], wv_t[:, 1, :],
                        start=False, stop=True,
                    )
                nc.vector.tensor_add(vt_sb[:, nt, :], v_ps[:, csl], bv_b[:])

        # ---- main attention loop ----
        # pac[j] accumulate P^T tiles off-PE (for the softmax rowsum)
        pac = [big.tile([128, MCHUNK], BF16, name=f"pac{j}" + r) for j in range(4)]

        pts = [None] * NT

        def emit_st_exp(nt):
            ksl = k_sb[:, nt * 128 : (nt + 1) * 128]
            st = pst.tile([128, MCHUNK], F32, tag="st", name=f"st{nt}" + r)
            pt = ptp.tile([128, MCHUNK], BF16, tag="pt", name=f"pt{nt}" + r)
            for mh in range(2):
                msl = slice(mh * 512, (mh + 1) * 512)
                nc.tensor.matmul(
                    st[:, msl], ksl, q_sb[:, msl], start=True, stop=True
                )
            nc.scalar.activation(pt[:], st[:], mybir.ActivationFunctionType.Exp)
            pts[nt] = pt
            # P-accumulation for the rowsum: chains 0,1 on DVE and 2,3 on
            # GPSIMD, so the DVE queue stays short (its latency feeds the
            # S^T psum-slot recycling chain).  The last tile (31) skips
            # accumulation entirely - the final rowsum matmul reads its
            # P^T tile directly, so nothing DVE-side gates the tail.
            if nt == NT - 1:
                return
            eng = nc.vector if (nt % 4 < 2 or nt == NT - 2) else nc.gpsimd
            if nt < 4:
                eng.tensor_copy(pac[nt], pt[:])
            else:
                eng.tensor_add(pac[nt % 4], pac[nt % 4], pt[:])
            if nt == NT - 3:
                # chains 0,1 complete at tiles 28,29; fold them early
                nc.vector.tensor_add(pac[0], pac[0], pac[1])
            if nt == NT - 2:
                # chain 3 ended at tile 27, chain 2 just got tile 30
                nc.vector.tensor_add(pac[2], pac[2], pac[3])

        def emit_pv(nt):
            first, last = nt == 0, nt == NT - 1
            pt = pts[nt]
            for h in range(2):
                vsl = vt_sb[:, nt, h * 128 : (h + 1) * 128]
                for mh in range(2):
                    msl = slice(mh * 512, (mh + 1) * 512)
                    nc.tensor.matmul(
                        acc[h][:, msl], vsl, pt[:, msl], start=first, stop=last
                    )

        # chunk-phased emission: project chunk i while attending chunk i-1
        LAG = 2
        for i in range(9):
            if i < 8:
                emit_proj_chunk(i)
            if i >= 1:
                for nt in range(4 * (i - 1), 4 * i):
                    emit_st_exp(nt)
                    if nt >= LAG:
                        emit_pv(nt - LAG)
        # rowsum/gamma = ginv^T(pac0+pac1) + ginv^T(pac2+pac3) +
        # ginv^T(pt31), accumulated in PSUM and interleaved with the PV
        # drain; the last matmul pair waits only on exp(31).
        rs_tile = pst.tile([128, MCHUNK], F32, tag="st", name="rs_t" + r)
        rs_ps = rs_tile[:1, :]
        emit_pv(NT - 2)
        for mh in range(2):
            msl = slice(mh * 512, (mh + 1) * 512)
            nc.tensor.matmul(
                rs_ps[:, msl], ginv_b[:], pac[0][:, msl], start=True, stop=False
            )
        emit_pv(NT - 1)
        for mh in range(2):
            msl = slice(mh * 512, (mh + 1) * 512)
            nc.tensor.matmul(
                rs_ps[:, msl], ginv_b[:], pac[2][:, msl], start=False, stop=False
            )
        for mh in range(2):
            msl = slice(mh * 512, (mh + 1) * 512)
            nc.tensor.matmul(
                rs_ps[:, msl], ginv_b[:], pts[NT - 1][:, msl],
                start=False, stop=True,
            )

        # ---- epilogue: out = acc * (gamma / rowsum) + x ----
        # recip on DVE, fp32 broadcast matmul on (idle) PE, ACT copies
        # the broadcast to SBUF (DVE cannot read two PSUM operands).
        rinv = epi.tile([1, MCHUNK], F32, name="rinv" + r)
        grecip_b = big.tile([128, MCHUNK], F32, name="gr_b" + r)
        # fp16 result tiles: halves the store traffic and the residual
        # add becomes all-2-byte (2x DVE); costs ~3e-3 absmax
        res = [
            epi.tile([128, MCHUNK], F16, tag=f"res{h}", name=f"res{h}" + r)
            for h in range(2)
        ]
        gr_tile = pst.tile([128, MCHUNK], F32, tag="st", name="gr_ps" + r)
        for mh in range(2):
            msl = slice(mh * 512, (mh + 1) * 512)
            nc.vector.reciprocal_approx_fast(rinv[:, msl], rs_ps[:, msl])
            nc.tensor.matmul(
                gr_tile[:, msl], ones_row[:], rinv[:, msl], start=True, stop=True
            )
            nc.scalar.copy(grecip_b[:, msl], gr_tile[:, msl])
            # GPSIMD cannot read PSUM: both muls (PSUM src) stay on DVE,
            # then the all-SBUF residual adds run DVE/GPSIMD in parallel.
            for h in range(2):
                nc.vector.tensor_mul(res[h][:, msl], acc[h][:, msl], grecip_b[:, msl])
            for h in range(2):
                eng = nc.vector if h == 0 else nc.gpsimd
                eng.tensor_add(res[h][:, msl], res[h][:, msl], xf_t[:, h, msl])
                nc.sync.dma_start(out[:, h, msl], res[h][:, msl])


def _build(repeats=1):
    nc = bacc.Bacc("TRN2", target_bir_lowering=False, debug=False, num_devices=NCORES)

    xf16 = nc.dram_tensor("xf16", [128, 2, N], F16, kind="ExternalInput")
    x8 = nc.dram_tensor("x8", [128, 2, N], F8 if V_FP8 else F16, kind="ExternalInput")
    wqk = nc.dram_tensor("wqk", [128, 2, 2 * C8], F16, kind="ExternalInput")
    wv8 = nc.dram_tensor(
        "wv8", [128, 2, C], F8 if V_FP8 else F16, kind="ExternalInput"
    )
    bqk = nc.dram_tensor("bqk", [C8, 2], F32, kind="ExternalInput")
    bv = nc.dram_tensor("bv", [1, C], F32, kind="ExternalInput")
    gamma = nc.dram_tensor("gamma", [1, 1], F32, kind="ExternalInput")
    out = nc.dram_tensor("out", [128, 2, MCHUNK], F16, kind="ExternalOutput")
    io = (xf16, x8, wqk, wv8, bqk, bv, gamma, out)

    with tile.TileContext(nc) as tc:
        for rep in range(repeats):
            _emit_body(nc, tc, io, rep)

    nc.compile()
    return nc


_NC_CACHE = {}


def _get_nc(repeats=1):
    if repeats not in _NC_CACHE:
        _NC_CACHE[repeats] = _build(repeats)
    return _NC_CACHE[repeats]


def _in_maps(x, Wq, bq, Wk, bk, Wv, bv, gamma):
    import ml_dtypes

    f8dt = ml_dtypes.float8_e4m3 if V_FP8 else np.float16
    xflat = x.reshape(2, C, N)
    # [C, N] -> [128, 2, N] with c = h*128 + p
    xh16 = xflat.astype(np.float16).reshape(2, 2, 128, N).transpose(0, 2, 1, 3)
    xh8 = xflat.astype(f8dt).reshape(2, 2, 128, N).transpose(0, 2, 1, 3)
    # [128, 2, 64]: wqk[p, h, o] = [Wq.T | Wk.T][h*128+p, o]
    wqk_full = np.concatenate([Wq.T, Wk.T], axis=1).astype(np.float16)  # [C, 64]
    wqk2 = np.ascontiguousarray(wqk_full.reshape(2, 128, 2 * C8).transpose(1, 0, 2))
    # [128, 2, 256]: wv8[p, h, co] = Wv[co, h*128+p]
    wv82 = np.ascontiguousarray(
        Wv.T.astype(f8dt).reshape(2, 128, C).transpose(1, 0, 2)
    )
    bqk2 = np.ascontiguousarray(
        np.stack([bq.reshape(C8), bk.reshape(C8)], axis=1).astype(np.float32)
    )
    bv2 = np.ascontiguousarray(bv.reshape(1, C))
    g2 = np.ascontiguousarray(gamma.reshape(1, 1))

    maps = []
    for core in range(NCORES):
        b, j = core // 4, core % 4
        roll = -j * MCHUNK
        maps.append(
            {
                "xf16": np.ascontiguousarray(np.roll(xh16[b], roll, axis=2)),
                "x8": np.ascontiguousarray(np.roll(xh8[b], roll, axis=2)),
                "wqk": wqk2,
                "wv8": wv82,
                "bqk": bqk2,
                "bv": bv2,
                "gamma": g2,
            }
        )
    return maps


def kernel(x, Wq, bq, Wk, bk, Wv, bv, gamma):
    global LAST_RESULTS
    x = np.ascontiguousarray(np.asarray(x, dtype=np.float32))
    args = [np.asarray(a, dtype=np.float32) for a in (Wq, bq, Wk, bk, Wv, bv, gamma)]

    B, Cc, D, H, W = x.shape
    assert (B, Cc, D * H * W) == (2, C, N), x.shape

    repeats = int(os.environ.get("ATTN_KERNEL_REPEATS", "1"))
    nc = _get_nc(repeats)
    maps = _in_maps(x, *args)
    kwargs = {}
    if int(os.environ.get("ATTN_KERNEL_TRACE", "0")):
        kwargs = dict(
            trace=True,
            trace_cores=[0],
            tmpdir=os.environ.get("ATTN_KERNEL_TRACE_DIR"),
        )
    res = run_bass_kernel_spmd(nc, maps, core_ids=list(range(NCORES)), **kwargs)
    LAST_RESULTS = res

    outf = np.empty((B, C, N), dtype=np.float32)
    for core in range(NCORES):
        b, j = core // 4, core % 4
        o = np.asarray(res.results[core]["out"], dtype=np.float32)  # [128, 2, 1024]
        outf[b][:, j * MCHUNK : (j + 1) * MCHUNK] = o.transpose(1, 0, 2).reshape(
            C, MCHUNK
        )
    return outf.reshape(B, Cc, D, H, W)
